# revision 18
# baseline (speedup 1.0000x reference)
"""Trainium2 Bass kernel for nn_CnnModel_70007966925195.

CNN backbone (3x conv1d+relu+maxpool2 -> mean -> FC+relu -> BN) followed by an
all-pairs contrastive loss. Data-parallel over N across 8 NeuronCores; z is
AllGathered and each core computes a 512x4096 row block of the loss matrix.

Strategy vs the bf16 baseline:
- conv2/conv3 matmuls run in fp8e4 with MatmulPerfMode.DoubleRow (K_eff=256,
  0.5 cyc/out-col): 3x fewer tensor cycles.  fp8 activation quantization is
  made accurate by CENTERING: evictions store a_l = h_l - r_l (r_l =
  per-channel reference mean, computed host-side from a sample subset), and
  the exact bias conv(r) is re-added inside the next eviction via per-
  partition scalars.  Interior positions share one bias; the 4 edge
  positions per layer get an exact per-position 3-op eviction.
- conv1 stays bf16 (x in fp8 costs ~1.3% final error): x stored as 32
  overlapping 128-row chunks at stride 16, so every output position pair is
  ONE matmul (shift folded into 16 weight variants).
- evictions: DVE max(psA,psB) -> bf16, then GPSIMD tensor_scalar
  max(m + (b-r), -r) == relu(m+b) - r -> fp8 ring (offloads to idle engine).
- loss: z pre-truncated to fp22 on device (bitwise_and) so a single fp32r
  z.z matmul is consistent with sq = |z|^2; d2 rank-2 term via one fp32
  K=2 matmul; y via bf16 K=2 matmul; eviction = sqrt + relu(1-d) + select.
"""

import os
import sys

try:
    import concourse.bass as bass  # noqa: F401
except ImportError:
    sys.path.insert(0, "/opt/trn_rl_repo")

import numpy as np
import ml_dtypes

import concourse.bass as bass  # noqa: F811
import concourse.mybir as mybir
import concourse.tile as tile
from concourse import bacc
from concourse.bass_utils import run_bass_kernel_spmd

F32 = mybir.dt.float32
F32R = mybir.dt.float32r
BF16 = mybir.dt.bfloat16
FP8 = mybir.dt.float8e4
AL = mybir.AluOpType
ACT = mybir.ActivationFunctionType
DR = mybir.MatmulPerfMode.DoubleRow

E4NP = ml_dtypes.float8_e4m3
BFNP = ml_dtypes.bfloat16

N_CORES = 8
N = 4096
NL = N // N_CORES   # 512 samples per core
L = 512
K1, C1 = 100, 64
K2, C2 = 5, 128
K3, C3 = 3, 256
T1 = 256            # pooled conv1 positions
T2 = 128            # pooled conv2 positions
T3 = 64             # pooled conv3 positions
W1P = 8             # h1 ring depth in pairs (must be even)
XCH = 32            # conv1 x chunks (128 rows, stride 16)

# cvec column indices
CV_NR1 = 0          # -r1 (tiled x2)
CV_B2I = 1          # b2 interior
CV_NR2 = 2          # -r2
CV_B2E = 3          # 3..6: b2[:, 0], b2[:, 1], b2[:, 254], b2[:, 255]
CV_B3I = 7          # 7,8: b3 interior ch0, ch1
CV_NR3 = 9          # 9,10: -r3 ch0, ch1
CV_B3E = 11         # 11..18: b3[:, (0,1,126,127)] x ch(0,1)
CV_K = 19

LAST_RESULT = None


def build_nc():
    kdebug = os.environ.get("KDEBUG", "full")
    nc = bacc.Bacc("TRN2", target_bir_lowering=False, debug=False,
                   num_devices=N_CORES)

    xs_d = nc.dram_tensor("xs", [XCH, 128, NL], BF16, kind="ExternalInput")
    w1s_d = nc.dram_tensor("w1s", [16, 128, 128], BF16, kind="ExternalInput")
    w2v_d = nc.dram_tensor("w2v", [8, 128, 2, 128], FP8, kind="ExternalInput")
    w3v_d = nc.dram_tensor("w3v", [8, 128, 2, 128], FP8, kind="ExternalInput")
    fcw_d = nc.dram_tensor("fcw", [2, 128, 128], BF16, kind="ExternalInput")
    fcb_d = nc.dram_tensor("fcb", [128, 1], F32, kind="ExternalInput")
    bna_d = nc.dram_tensor("bna", [128, 1], F32, kind="ExternalInput")
    bnb_d = nc.dram_tensor("bnb", [128, 1], F32, kind="ExternalInput")
    cvec_d = nc.dram_tensor("cvec", [128, CV_K], F32, kind="ExternalInput")
    abl_d = nc.dram_tensor("abl", [2, NL], BF16, kind="ExternalInput")
    abf_d = nc.dram_tensor("abf", [2, N], BF16, kind="ExternalInput")
    onc_d = nc.dram_tensor("onc", [128, 1], F32, kind="ExternalInput")
    onr_d = nc.dram_tensor("onr", [1, N], F32, kind="ExternalInput")
    out_d = nc.dram_tensor("out", [NL, N], F32, kind="ExternalOutput")
    gin_d = nc.dram_tensor("gin", [129, NL], F32, kind="Internal")
    gout_d = nc.dram_tensor("gout", [N_CORES, 129, NL], F32, kind="Internal",
                            addr_space="Shared")

    with tile.TileContext(nc) as tc:
        with (
            tc.tile_pool(name="const", bufs=1) as cpool,
            tc.tile_pool(name="zbuf", bufs=1) as zpool,
            tc.tile_pool(name="fcp", bufs=1, space="PSUM") as fcpool,
        ):
            # ---- persistent SBUF tensors ----
            xs = cpool.tile([128, XCH, NL], BF16, tag="xs")
            nc.sync.dma_start(xs[:], xs_d[:].rearrange("c p n -> p c n"))
            w1s = cpool.tile([128, 16, 128], BF16, tag="w1s")
            nc.sync.dma_start(w1s[:], w1s_d[:].rearrange("s k o -> k s o"))
            w2v = cpool.tile([128, 8, 2, 128], FP8, tag="w2v")
            nc.sync.dma_start(w2v[:], w2v_d[:].rearrange("v k t o -> k v t o"))
            w3v = cpool.tile([128, 8, 2, 128], FP8, tag="w3v")
            nc.sync.dma_start(w3v[:], w3v_d[:].rearrange("v k t o -> k v t o"))
            fcw = cpool.tile([128, 2, 128], BF16, tag="fcw")
            nc.sync.dma_start(fcw[:], fcw_d[:].rearrange("c k o -> k c o"))
            fcb = cpool.tile([128, 1], F32, tag="fcb")
            nc.sync.dma_start(fcb[:], fcb_d[:])
            bna = cpool.tile([128, 1], F32, tag="bna")
            nc.sync.dma_start(bna[:], bna_d[:])
            bnb = cpool.tile([128, 1], F32, tag="bnb")
            nc.sync.dma_start(bnb[:], bnb_d[:])
            cvec = cpool.tile([128, CV_K], F32, tag="cvec")
            nc.sync.dma_start(cvec[:], cvec_d[:])
            abl = cpool.tile([2, NL], BF16, tag="abl")
            nc.sync.dma_start(abl[:], abl_d[:])
            abf = cpool.tile([2, N], BF16, tag="abf")
            nc.sync.dma_start(abf[:], abf_d[:])

            # rings / stores
            h1r = cpool.tile([128, W1P, NL], FP8, tag="h1r")   # pair-slots
            h2r = cpool.tile([128, T2 // 2, 2, NL], FP8, tag="h2r")  # full
            fc_ps = fcpool.tile([128, NL], F32, tag="fc")

            with (
                tc.tile_pool(name="p1", bufs=1, space="PSUM") as p1pool,
                tc.tile_pool(name="p2", bufs=1, space="PSUM") as p2pool,
                tc.tile_pool(name="p3", bufs=1, space="PSUM") as p3pool,
                tc.tile_pool(name="h3", bufs=2) as h3pool,
                tc.tile_pool(name="ev", bufs=4) as evpool,
            ):
                def conv1_batch(i1):
                    # bank j: cols 0-63 pos 4i1+j, 64-127 pos 4i1+j+2
                    ps = p1pool.tile([128, 2, NL], F32, tag="p1")
                    for j in range(2):
                        la = 4 * i1 + j
                        c, s = divmod(la, 16)
                        nc.tensor.matmul(ps[:, j, :], w1s[:, s, :],
                                         xs[:, c, :], start=True, stop=True)
                    # 3-op eviction: ACT stages bank0 (one PSUM src per op),
                    # DVE folds bank1 + relu, GPSIMD centers into the ring.
                    ta = evpool.tile([128, NL], F32, tag="evA")
                    nc.scalar.activation(ta[:], ps[:, 0, :], ACT.Relu)
                    m2 = evpool.tile([128, NL], F32, tag="evB")
                    nc.vector.scalar_tensor_tensor(
                        m2[:], ps[:, 1, :], 0.0, ta[:],
                        op0=AL.max, op1=AL.max)
                    nc.gpsimd.tensor_scalar_add(
                        h1r[:, i1 % W1P, :], m2[:],
                        cvec[:, CV_NR1:CV_NR1 + 1])

                def conv2_pair(j2):
                    ps = p2pool.tile([128, 2, NL], F32, tag="p2")
                    for jj in range(2):
                        l2 = 2 * j2 + jj
                        m4 = l2 % 4
                        ua = (l2 - 2) // 4
                        mlist = []
                        if ua >= 0:
                            mlist.append((ua, m4))          # role A variant
                        if ua + 1 <= (T1 // 4) - 1:
                            mlist.append((ua + 1, 4 + m4))  # role B variant
                        for ti, (u, v) in enumerate(mlist):
                            nc.tensor.matmul(
                                ps[:, jj, :], w2v[:, v, :, :],
                                h1r[:, (2 * u) % W1P:(2 * u) % W1P + 2, :],
                                start=(ti == 0), stop=(ti == len(mlist) - 1),
                                perf_mode=DR)
                    slot = h2r[:, j2 // 2, j2 % 2, :]
                    if j2 == 0:
                        ca, cb = CV_B2E, CV_B2E + 1
                    elif j2 == T2 - 1:
                        ca, cb = CV_B2E + 2, CV_B2E + 3
                    else:
                        ca = cb = CV_B2I
                    ta = evpool.tile([128, NL], F32, tag="evA")
                    nc.scalar.activation(ta[:], ps[:, 0, :], ACT.Relu,
                                         bias=cvec[:, ca:ca + 1], scale=1.0)
                    m2 = evpool.tile([128, NL], F32, tag="evB")
                    nc.vector.scalar_tensor_tensor(
                        m2[:], ps[:, 1, :], cvec[:, cb:cb + 1], ta[:],
                        op0=AL.add, op1=AL.max)
                    nc.gpsimd.tensor_scalar_add(
                        slot, m2[:], cvec[:, CV_NR2:CV_NR2 + 1])

                def conv3_pair(j3):
                    h3t = h3pool.tile([128, 2, NL], BF16, tag="h3t")
                    for ch in range(2):
                        ps = p3pool.tile([128, 2, NL], F32, tag="p3")
                        for jj in range(2):
                            l3 = 2 * j3 + jj
                            par = l3 % 2
                            pa = (l3 - 1) // 2
                            mlist = []
                            if pa >= 0:
                                mlist.append((pa, 4 * par + 2 * ch))
                            if pa + 1 <= T2 // 2 - 1:
                                mlist.append((pa + 1, 4 * par + 2 * ch + 1))
                            for ti, (p, v) in enumerate(mlist):
                                nc.tensor.matmul(
                                    ps[:, jj, :], w3v[:, v, :, :],
                                    h2r[:, p, :, :],
                                    start=(ti == 0),
                                    stop=(ti == len(mlist) - 1),
                                    perf_mode=DR)
                        if j3 == 0:
                            ca = CV_B3E + 4 * ch
                            cb = ca + 1
                        elif j3 == T3 - 1:
                            ca = CV_B3E + 4 * ch + 2
                            cb = ca + 1
                        else:
                            ca = cb = CV_B3I + ch
                        ta = evpool.tile([128, NL], F32, tag="evA")
                        nc.scalar.activation(ta[:], ps[:, 0, :], ACT.Relu,
                                             bias=cvec[:, ca:ca + 1],
                                             scale=1.0)
                        m2 = evpool.tile([128, NL], F32, tag="evB")
                        nc.vector.scalar_tensor_tensor(
                            m2[:], ps[:, 1, :], cvec[:, cb:cb + 1], ta[:],
                            op0=AL.add, op1=AL.max)
                        nc.gpsimd.tensor_scalar_add(
                            h3t[:, ch, :], m2[:],
                            cvec[:, CV_NR3 + ch:CV_NR3 + ch + 1])
                    for ch in range(2):
                        nc.tensor.matmul(
                            fc_ps[:], fcw[:, ch, :], h3t[:, ch, :],
                            start=(j3 == 0 and ch == 0),
                            stop=(j3 == T3 - 1 and ch == 1),
                            skip_group_check=True)

                kph = os.environ.get("KPHASES", "123")
                for ii in range(138):
                    if ii < 128 and "1" in kph:
                        conv1_batch(ii)
                    if 6 <= ii < 134 and "2" in kph:
                        conv2_pair(ii - 6)
                    if (ii >= 9 and (ii - 9) % 2 == 0 and (ii - 9) // 2 < T3
                            and "3" in kph):
                        conv3_pair((ii - 9) // 2)
                if "3" not in kph:
                    nc.tensor.matmul(fc_ps[:], fcw[:, 0, :],
                                     xs[:, 0, 0:NL],
                                     start=True, stop=True)

            if kdebug == "convs":
                dbg = zpool.tile([128, NL], F32, tag="zT")
                nc.vector.tensor_copy(dbg[:], fc_ps[:])
                nc.sync.dma_start(out_d[0:128, 0:NL], dbg[:])
            else:
                _emit_tail(nc, tc, zpool, fc_ps, fcb, bna, bnb, abl, abf,
                           onc_d, onr_d, out_d, gin_d, gout_d, kdebug)

    nc.compile()
    return nc


def _emit_tail(nc, tc, zpool, fc_ps, fcb, bna, bnb, abl, abf,
               onc_d, onr_d, out_d, gin_d, gout_d, kdebug):
    I32 = mybir.dt.int32
    with tc.tile_pool(name="sqp", bufs=1, space="PSUM") as sqpool:
        zT = zpool.tile([128, NL], F32, tag="zT")
        nc.scalar.activation(zT[:], fc_ps[:], ACT.Relu,
                             bias=fcb[:], scale=1.0)
        nc.vector.tensor_scalar(zT[:], zT[:], bna[:], bnb[:],
                                op0=AL.mult, op1=AL.add)
        # truncate z to fp22 (e8m13) so the fp32r z.z matmul is consistent
        # with sq computed from the same values
        nc.vector.tensor_scalar(zT[:].bitcast(I32), zT[:].bitcast(I32),
                                -1024, None, op0=AL.bitwise_and,
                                op1=AL.bypass)
        zsq = zpool.tile([128, NL], F32, tag="zsq")
        nc.scalar.activation(zsq[:], zT[:], ACT.Square)
        ones_col = zpool.tile([128, 1], F32, tag="ones_col")
        nc.sync.dma_start(ones_col[:], onc_d[:])
        sq_ps = sqpool.tile([1, NL], F32, tag="sq")
        nc.tensor.matmul(sq_ps[:], ones_col[:], zsq[:],
                         start=True, stop=True)
        sqones = zpool.tile([2, NL], F32, tag="sqones")
        nc.sync.dma_start(sqones[1:2, :], onr_d[0:1, 0:NL])
        nc.vector.tensor_copy(sqones[0:1, :], sq_ps[:])
        zm2 = zpool.tile([128, NL], F32R, tag="zm2")
        nc.vector.tensor_scalar_mul(zm2[:], zT[:], -2.0)

        if kdebug == "z":
            nc.sync.dma_start(out_d[0:128, 0:NL], zT[:])
            return

        nc.sync.dma_start(gin_d[0:128, :], zT[:])
        nc.sync.dma_start(gin_d[128:129, :], sqones[0:1, :])
        nc.gpsimd.collective_compute(
            "AllGather", AL.bypass,
            replica_groups=[list(range(N_CORES))],
            ins=[gin_d[:]], outs=[gout_d[:]],
        )

        zfT = zpool.tile([128, N_CORES, NL], F32R, tag="zfT")
        nc.sync.dma_start(
            zfT[:],
            gout_d[:, 0:128, :].rearrange("r p n -> p r n").bitcast(F32R))
        onesqf = zpool.tile([2, N], F32, tag="onesqf")
        nc.sync.dma_start(onesqf[0:1, :], onr_d[:])
        nc.sync.dma_start(
            onesqf[1:2, :].rearrange("p (r n) -> p r n", r=N_CORES),
            gout_d[:, 128:129, :].rearrange("r p n -> p r n"))

        if kdebug == "gather":
            zfc = zpool.tile([128, NL], F32, tag="zfc")
            nc.vector.tensor_copy(zfc[:], zfT[:, 0, :])
            nc.sync.dma_start(out_d[0:128, 0:NL], zfc[:])
            return

        # ---- loss row block ----
        with (
            tc.tile_pool(name="pd", bufs=2, space="PSUM") as pdpool,
            tc.tile_pool(name="py", bufs=2, space="PSUM") as pypool,
            tc.tile_pool(name="lw", bufs=4) as lwpool,
        ):
            for rb in range(4):
                rs = slice(128 * rb, 128 * rb + 128)
                for jc in range(N_CORES):
                    js = slice(NL * jc, NL * jc + NL)
                    pd = pdpool.tile([128, NL], F32, tag="pd")
                    py = pypool.tile([128, NL], F32, tag="py")
                    nc.tensor.matmul(pd[:], zm2[:, rs], zfT[:, jc, :],
                                     start=True, stop=False)
                    nc.tensor.matmul(pd[:], sqones[:, rs], onesqf[:, js],
                                     start=False, stop=True)
                    nc.tensor.matmul(py[:], abl[:, rs], abf[:, js],
                                     start=True, stop=True)
                    dd = lwpool.tile([128, NL], F32, tag="dd")
                    nc.scalar.activation(dd[:], pd[:], ACT.Sqrt)
                    tt = lwpool.tile([128, NL], F32, tag="tt")
                    nc.scalar.activation(tt[:], dd[:], ACT.Relu,
                                         bias=1.0, scale=-1.0)
                    cl = lwpool.tile([128, NL], F32, tag="cl")
                    nc.vector.select(
                        cl[:], py[:].bitcast(I32), dd[:], tt[:])
                    nc.sync.dma_start(out_d[rs, js], cl[:])


def _prep_inputs(samples, samples_info, conv1_w, conv1_b, conv2_w, conv2_b,
                 conv3_w, conv3_b, fc_w, fc_b, bn_gamma, bn_beta, bn_mean,
                 bn_var):
    f = np.float32
    samples = np.asarray(samples, f)
    info = np.asarray(samples_info, f)
    w1 = np.asarray(conv1_w, f)
    w2 = np.asarray(conv2_w, f)
    w3 = np.asarray(conv3_w, f)
    fcwf = np.asarray(fc_w, f)

    assert np.all(np.asarray(conv1_b) == 0), "conv1_b != 0 unsupported"
    assert np.all(np.asarray(conv2_b) == 0), "conv2_b != 0 unsupported"
    assert np.all(np.asarray(conv3_b) == 0), "conv3_b != 0 unsupported"

    # ---- reference means r_l from a sample subset (fp32 chain) ----
    nsub = 256
    xs_sub = samples[:nsub, 0, :]                       # [S, L]

    def convl(h, w, pad):
        # h [S, C, Lc], w [O, C, K] -> [S, O, Lc]
        S, C, Lc = h.shape
        O, _, K = w.shape
        hp = np.pad(h, ((0, 0), (0, 0), pad))
        cols = np.stack([hp[:, :, k:k + Lc] for k in range(K)], axis=1)
        cols = cols.reshape(S, K * C, Lc)
        wm = w.transpose(0, 2, 1).reshape(O, K * C)
        return np.einsum('ok,nkl->nol', wm, cols, optimize=True)

    def pool2(x):
        n, c, l = x.shape
        return x.reshape(n, c, l // 2, 2).max(-1)

    h1s = np.maximum(pool2(convl(xs_sub[:, None, :], w1, (49, 50))), 0)
    r1 = h1s.mean(axis=(0, 2)).astype(f)                # [64]
    h2s = np.maximum(pool2(convl(h1s, w2, (2, 2))), 0)
    r2 = h2s.mean(axis=(0, 2)).astype(f)                # [128]
    h3s = np.maximum(pool2(convl(h2s, w3, (1, 1))), 0)
    r3 = h3s.mean(axis=(0, 2)).astype(f)                # [256]

    # exact per-position biases conv(r) at the pre-pool level
    # conv2 input length = T1 (pooled conv1), conv3 input length = T2
    b2 = convl(np.broadcast_to(r1[None, :, None], (1, C1, T1)).astype(f),
               w2, (2, 2))[0]                           # [128, T1]
    b3 = convl(np.broadcast_to(r2[None, :, None], (1, C2, T2)).astype(f),
               w3, (1, 1))[0]                           # [256, T2]
    b2i = b2[:, 100]                                    # interior value
    b3i = b3[:, 100]

    # ---- conv1 shifted weights: 16 variants, window <= 118 rows ----
    w1s = np.zeros((16, 128, 128), f)
    for s in range(16):
        w1s[s, s:s + K1, 0:C1] = w1[:, 0, :].T
        w1s[s, s + 2:s + 2 + K1, C1:128] = w1[:, 0, :].T

    # ---- conv2 DR weight variants: v = role*4 + (l2 % 4) ----
    # contraction row (p, t): pooled pi = 4u + 2t + p//64, ic = p % 64
    # tap tau = pi - (l2 - 2); role A: l2-4u = m4+4, role B: l2-4u = m4
    w2v = np.zeros((8, 128, 2, C2), f)
    for role in range(2):
        for m4 in range(4):
            # uA = (l2-2)//4 -> deltaA = l2-4uA = [4,5,2,3][m4]; deltaB-4
            delta = (m4 + 4 if m4 < 2 else m4) - (0 if role == 0 else 4)
            for t in range(2):
                for half in range(2):
                    tau = 2 * t + half - delta + 2
                    if 0 <= tau < K2:
                        w2v[role * 4 + m4, 64 * half:64 * half + 64, t, :] = \
                            w2[:, :, tau].T
    # ---- conv3 DR variants: v = 4*parity + 2*ch + role ----
    # rows = 128 ic; ktile t covers pooled2 pi = 2u + t; tau = pi - (l3-1)
    # even l3 (par 0): role A u=m-1 -> tau = t-1 ; role B u=m -> tau = t+1
    # odd  l3 (par 1): role A u=m   -> tau = t   ; role B u=m+1 -> tau = t+2
    w3v = np.zeros((8, 128, 2, 128), f)
    for par in range(2):
        for role in range(2):
            for ch in range(2):
                for t in range(2):
                    tau = t + [[-1, 1], [0, 2]][par][role]
                    if 0 <= tau < K3:
                        w3v[4 * par + 2 * ch + role, :, t, :] = \
                            w3[128 * ch:128 * ch + 128, :, tau].T

    fcw = np.zeros((2, 128, 128), f)
    fcwT = fcwf.T / f(T3)
    fcw[0] = fcwT[0:128, :]
    fcw[1] = fcwT[128:256, :]
    fcb = (np.asarray(fc_b, f) + fcwf @ r3).reshape(128, 1)
    bna = (np.asarray(bn_gamma, f) /
           np.sqrt(np.asarray(bn_var, f) + f(1e-5))).reshape(128, 1)
    bnb = (np.asarray(bn_beta, f) -
           np.asarray(bn_mean, f).reshape(128) * bna[:, 0]).reshape(128, 1)

    # ---- cvec ----
    cvec = np.zeros((128, CV_K), f)
    cvec[:, CV_NR1] = -np.tile(r1, 2)
    cvec[:, CV_B2I] = b2i
    cvec[:, CV_NR2] = -r2
    for k, l2 in enumerate((0, 1, T1 - 2, T1 - 1)):
        cvec[:, CV_B2E + k] = b2[:, l2]
    for ch in range(2):
        cs = slice(128 * ch, 128 * ch + 128)
        cvec[:, CV_B3I + ch] = b3i[cs]
        cvec[:, CV_NR3 + ch] = -r3[cs]
        for k, l3 in enumerate((0, 1, T2 - 2, T2 - 1)):
            cvec[:, CV_B3E + 4 * ch + k] = b3[cs, l3]

    writer, gen = info[:, 0], info[:, 1]
    assert np.all((writer == 0) | (writer == 1)), "non-binary writer id"
    a_full = (gen * (1.0 - writer)).astype(f)
    b_full = (gen * writer).astype(f)
    abf = np.stack([a_full, b_full])

    w1s_b = w1s.astype(BFNP)
    w2v_8 = w2v.astype(E4NP)
    w3v_8 = w3v.astype(E4NP)
    fcw_b = fcw.astype(BFNP)

    ones_col_np = np.ones((128, 1), f)
    ones_row_np = np.ones((1, N), f)

    in_maps = []
    for core in range(N_CORES):
        n0 = core * NL
        xpad = np.zeros((624, NL), f)
        xpad[49:49 + L, :] = samples[n0:n0 + NL, 0, :].T
        xsc = np.zeros((XCH, 128, NL), f)
        for c in range(XCH):
            xsc[c] = xpad[16 * c:16 * c + 128, :]
        in_maps.append({
            "xs": xsc.astype(BFNP), "onc": ones_col_np, "onr": ones_row_np,
            "w1s": w1s_b, "w2v": w2v_8, "w3v": w3v_8, "fcw": fcw_b,
            "fcb": fcb, "bna": bna, "bnb": bnb, "cvec": cvec,
            "abl": np.ascontiguousarray(abf[:, n0:n0 + NL]).astype(BFNP),
            "abf": abf.astype(BFNP),
        })
    return in_maps


def kernel(**inputs):
    global LAST_RESULT
    in_maps = _prep_inputs(**inputs)
    nc = build_nc()
    res = run_bass_kernel_spmd(nc, in_maps, core_ids=list(range(N_CORES)))
    LAST_RESULT = res
    out = np.concatenate([r["out"] for r in res.results], axis=0)
    np.fill_diagonal(out, 0.0)
    return out.astype(np.float32)


# revision 19
# speedup vs baseline: 3.1779x; 3.1779x over previous
"""Trainium2 Bass kernel for nn_CnnModel_70007966925195.

CNN backbone (3x conv1d+relu+maxpool2 -> mean -> FC+relu -> BN) followed by an
all-pairs contrastive loss. Data-parallel over N across 8 NeuronCores; z is
AllGathered and each core computes a 512x4096 row block of the loss matrix.

Strategy vs the bf16 baseline:
- conv2/conv3 matmuls run in fp8e4 with MatmulPerfMode.DoubleRow (K_eff=256,
  0.5 cyc/out-col): 3x fewer tensor cycles.  fp8 activation quantization is
  made accurate by CENTERING: evictions store a_l = h_l - r_l (r_l =
  per-channel reference mean, computed host-side from a sample subset), and
  the exact bias conv(r) is re-added inside the next eviction via per-
  partition scalars.  Interior positions share one bias; the 4 edge
  positions per layer get an exact per-position 3-op eviction.
- conv1 stays bf16 (x in fp8 costs ~1.3% final error): x stored as 32
  overlapping 128-row chunks at stride 16, so every output position pair is
  ONE matmul (shift folded into 16 weight variants).
- evictions: DVE max(psA,psB) -> bf16, then GPSIMD tensor_scalar
  max(m + (b-r), -r) == relu(m+b) - r -> fp8 ring (offloads to idle engine).
- loss: z pre-truncated to fp22 on device (bitwise_and) so a single fp32r
  z.z matmul is consistent with sq = |z|^2; d2 rank-2 term via one fp32
  K=2 matmul; y via bf16 K=2 matmul; eviction = sqrt + relu(1-d) + select.
"""

import os
import sys

try:
    import concourse.bass as bass  # noqa: F401
except ImportError:
    sys.path.insert(0, "/opt/trn_rl_repo")

import numpy as np
import ml_dtypes

import concourse.bass as bass  # noqa: F811
import concourse.mybir as mybir
import concourse.tile as tile
from concourse import bacc
from concourse.bass_utils import run_bass_kernel_spmd

F32 = mybir.dt.float32
F32R = mybir.dt.float32r
BF16 = mybir.dt.bfloat16
FP8 = mybir.dt.float8e4
AL = mybir.AluOpType
ACT = mybir.ActivationFunctionType
DR = mybir.MatmulPerfMode.DoubleRow

E4NP = ml_dtypes.float8_e4m3
BFNP = ml_dtypes.bfloat16

N_CORES = 8
N = 4096
NL = N // N_CORES   # 512 samples per core
L = 512
K1, C1 = 100, 64
K2, C2 = 5, 128
K3, C3 = 3, 256
T1 = 256            # pooled conv1 positions
T2 = 128            # pooled conv2 positions
T3 = 64             # pooled conv3 positions
W1P = 8             # h1 ring depth in pairs (must be even)
XCH = 32            # conv1 x chunks (128 rows, stride 16)

# cvec column indices (stage biases already include -r where centering)
CV_NR1 = 0          # -r1 (conv1 stage bias AND clamp)
CV_B2I = 1          # b2_int - r2 (conv2 interior stage bias)
CV_NR2 = 2          # -r2 (conv2 clamp)
CV_B2E = 3          # 3..6: b2[:, (0,1,254,255)] - r2
CV_B3I = 7          # 7,8: b3_int ch0, ch1 (raw; conv3 not centered)
CV_B3E = 9          # 9..16: b3[:, (0,1,126,127)] x ch, raw
CV_K = 17

LAST_RESULT = None


def build_nc():
    kdebug = os.environ.get("KDEBUG", "full")
    nc = bacc.Bacc("TRN2", target_bir_lowering=False, debug=False,
                   num_devices=N_CORES)

    xs_d = nc.dram_tensor("xs", [XCH, 128, NL], BF16, kind="ExternalInput")
    w1s_d = nc.dram_tensor("w1s", [16, 128, 128], BF16, kind="ExternalInput")
    w2v_d = nc.dram_tensor("w2v", [8, 128, 2, 128], FP8, kind="ExternalInput")
    w3v_d = nc.dram_tensor("w3v", [8, 128, 2, 128], FP8, kind="ExternalInput")
    fcw_d = nc.dram_tensor("fcw", [2, 128, 128], BF16, kind="ExternalInput")
    fcb_d = nc.dram_tensor("fcb", [128, 1], F32, kind="ExternalInput")
    bna_d = nc.dram_tensor("bna", [128, 1], F32, kind="ExternalInput")
    bnb_d = nc.dram_tensor("bnb", [128, 1], F32, kind="ExternalInput")
    cvec_d = nc.dram_tensor("cvec", [128, CV_K], F32, kind="ExternalInput")
    abl_d = nc.dram_tensor("abl", [2, NL], BF16, kind="ExternalInput")
    abf_d = nc.dram_tensor("abf", [2, N], BF16, kind="ExternalInput")
    onc_d = nc.dram_tensor("onc", [128, 1], F32, kind="ExternalInput")
    onr_d = nc.dram_tensor("onr", [1, N], F32, kind="ExternalInput")
    out_d = nc.dram_tensor("out", [NL, N], F32, kind="ExternalOutput")
    gin_d = nc.dram_tensor("gin", [129, NL], F32, kind="Internal")
    gout_d = nc.dram_tensor("gout", [N_CORES, 129, NL], F32, kind="Internal",
                            addr_space="Shared")

    with tile.TileContext(nc) as tc:
        with (
            tc.tile_pool(name="const", bufs=1) as cpool,
            tc.tile_pool(name="zbuf", bufs=1) as zpool,
            tc.tile_pool(name="fcp", bufs=1, space="PSUM") as fcpool,
        ):
            # ---- persistent SBUF tensors ----
            xs = cpool.tile([128, XCH, NL], BF16, tag="xs")
            nc.sync.dma_start(xs[:], xs_d[:].rearrange("c p n -> p c n"))
            w1s = cpool.tile([128, 16, 128], BF16, tag="w1s")
            nc.sync.dma_start(w1s[:], w1s_d[:].rearrange("s k o -> k s o"))
            w2v = cpool.tile([128, 8, 2, 128], FP8, tag="w2v")
            nc.sync.dma_start(w2v[:], w2v_d[:].rearrange("v k t o -> k v t o"))
            w3v = cpool.tile([128, 8, 2, 128], FP8, tag="w3v")
            nc.sync.dma_start(w3v[:], w3v_d[:].rearrange("v k t o -> k v t o"))
            fcw = cpool.tile([128, 2, 128], BF16, tag="fcw")
            nc.sync.dma_start(fcw[:], fcw_d[:].rearrange("c k o -> k c o"))
            fcb = cpool.tile([128, 1], F32, tag="fcb")
            nc.sync.dma_start(fcb[:], fcb_d[:])
            bna = cpool.tile([128, 1], F32, tag="bna")
            nc.sync.dma_start(bna[:], bna_d[:])
            bnb = cpool.tile([128, 1], F32, tag="bnb")
            nc.sync.dma_start(bnb[:], bnb_d[:])
            cvec = cpool.tile([128, CV_K], F32, tag="cvec")
            nc.sync.dma_start(cvec[:], cvec_d[:])
            abl = cpool.tile([2, NL], BF16, tag="abl")
            nc.sync.dma_start(abl[:], abl_d[:])
            abf = cpool.tile([2, N], BF16, tag="abf")
            nc.sync.dma_start(abf[:], abf_d[:])

            # rings / stores
            h1r = cpool.tile([128, W1P, NL], FP8, tag="h1r")   # pair-slots
            h2r = cpool.tile([128, T2 // 2, 2, NL], FP8, tag="h2r")  # full
            fc_ps = fcpool.tile([128, NL], F32, tag="fc")

            with (
                tc.tile_pool(name="p1", bufs=1, space="PSUM") as p1pool,
                tc.tile_pool(name="p2", bufs=1, space="PSUM") as p2pool,
                tc.tile_pool(name="p3", bufs=1, space="PSUM") as p3pool,
                tc.tile_pool(name="h3", bufs=2) as h3pool,
                tc.tile_pool(name="ev", bufs=4) as evpool,
            ):
                def conv1_batch(i1):
                    # bank j: cols 0-63 pos 4i1+j, 64-127 pos 4i1+j+2
                    ps = p1pool.tile([128, 2, NL], F32, tag="p1")
                    for j in range(2):
                        la = 4 * i1 + j
                        c, s = divmod(la, 16)
                        nc.tensor.matmul(ps[:, j, :], w1s[:, s, :],
                                         xs[:, c, :], start=True, stop=True)
                    # 2-op eviction: stage = Identity(ps + (b-r)) over both
                    # banks at once; combine = max(tA, clamp) max tB -> ring.
                    tab = evpool.tile([128, 2, NL], F32, tag="evA")
                    nc.scalar.activation(tab[:], ps[:, :, :], ACT.Identity,
                                         bias=cvec[:, CV_NR1:CV_NR1 + 1],
                                         scale=1.0)
                    nc.vector.scalar_tensor_tensor(
                        h1r[:, i1 % W1P, :], tab[:, 0, :],
                        cvec[:, CV_NR1:CV_NR1 + 1], tab[:, 1, :],
                        op0=AL.max, op1=AL.max)

                def conv2_pair(j2):
                    ps = p2pool.tile([128, 2, NL], F32, tag="p2")
                    for jj in range(2):
                        l2 = 2 * j2 + jj
                        m4 = l2 % 4
                        ua = (l2 - 2) // 4
                        mlist = []
                        if ua >= 0:
                            mlist.append((ua, m4))          # role A variant
                        if ua + 1 <= (T1 // 4) - 1:
                            mlist.append((ua + 1, 4 + m4))  # role B variant
                        for ti, (u, v) in enumerate(mlist):
                            nc.tensor.matmul(
                                ps[:, jj, :], w2v[:, v, :, :],
                                h1r[:, (2 * u) % W1P:(2 * u) % W1P + 2, :],
                                start=(ti == 0), stop=(ti == len(mlist) - 1),
                                perf_mode=DR)
                    slot = h2r[:, j2 // 2, j2 % 2, :]
                    tab = evpool.tile([128, 2, NL], F32, tag="evA")
                    if j2 in (0, T2 - 1):
                        ca = CV_B2E + (0 if j2 == 0 else 2)
                        nc.scalar.activation(tab[:, 0, :], ps[:, 0, :],
                                             ACT.Identity,
                                             bias=cvec[:, ca:ca + 1],
                                             scale=1.0)
                        nc.scalar.activation(tab[:, 1, :], ps[:, 1, :],
                                             ACT.Identity,
                                             bias=cvec[:, ca + 1:ca + 2],
                                             scale=1.0)
                    elif j2 % 2 == 0:
                        nc.scalar.activation(tab[:], ps[:, :, :],
                                             ACT.Identity,
                                             bias=cvec[:, CV_B2I:CV_B2I + 1],
                                             scale=1.0)
                    else:
                        nc.vector.tensor_scalar_add(
                            tab[:], ps[:, :, :], cvec[:, CV_B2I:CV_B2I + 1])
                    nc.vector.scalar_tensor_tensor(
                        slot, tab[:, 0, :], cvec[:, CV_NR2:CV_NR2 + 1],
                        tab[:, 1, :], op0=AL.max, op1=AL.max)

                def conv3_pair(j3):
                    h3t = h3pool.tile([128, 2, NL], BF16, tag="h3t")
                    for ch in range(2):
                        ps = p3pool.tile([128, 2, NL], F32, tag="p3")
                        for jj in range(2):
                            l3 = 2 * j3 + jj
                            par = l3 % 2
                            pa = (l3 - 1) // 2
                            mlist = []
                            if pa >= 0:
                                mlist.append((pa, 4 * par + 2 * ch))
                            if pa + 1 <= T2 // 2 - 1:
                                mlist.append((pa + 1, 4 * par + 2 * ch + 1))
                            for ti, (p, v) in enumerate(mlist):
                                nc.tensor.matmul(
                                    ps[:, jj, :], w3v[:, v, :, :],
                                    h2r[:, p, :, :],
                                    start=(ti == 0),
                                    stop=(ti == len(mlist) - 1),
                                    perf_mode=DR)
                        tab = evpool.tile([128, 2, NL], F32, tag="evA")
                        if j3 in (0, T3 - 1):
                            ca = CV_B3E + 4 * ch + (0 if j3 == 0 else 2)
                            nc.scalar.activation(tab[:, 0, :], ps[:, 0, :],
                                                 ACT.Identity,
                                                 bias=cvec[:, ca:ca + 1],
                                                 scale=1.0)
                            nc.scalar.activation(tab[:, 1, :], ps[:, 1, :],
                                                 ACT.Identity,
                                                 bias=cvec[:, ca + 1:ca + 2],
                                                 scale=1.0)
                        else:
                            cc = CV_B3I + ch
                            nc.scalar.activation(tab[:], ps[:, :, :],
                                                 ACT.Identity,
                                                 bias=cvec[:, cc:cc + 1],
                                                 scale=1.0)
                        nc.vector.scalar_tensor_tensor(
                            h3t[:, ch, :], tab[:, 0, :], 0.0, tab[:, 1, :],
                            op0=AL.max, op1=AL.max)
                    for ch in range(2):
                        nc.tensor.matmul(
                            fc_ps[:], fcw[:, ch, :], h3t[:, ch, :],
                            start=(j3 == 0 and ch == 0),
                            stop=(j3 == T3 - 1 and ch == 1),
                            skip_group_check=True)

                kph = os.environ.get("KPHASES", "123")
                for ii in range(138):
                    if ii < 128 and "1" in kph:
                        conv1_batch(ii)
                    if 6 <= ii < 134 and "2" in kph:
                        conv2_pair(ii - 6)
                    if (ii >= 9 and (ii - 9) % 2 == 0 and (ii - 9) // 2 < T3
                            and "3" in kph):
                        conv3_pair((ii - 9) // 2)
                if "3" not in kph:
                    nc.tensor.matmul(fc_ps[:], fcw[:, 0, :],
                                     xs[:, 0, 0:NL],
                                     start=True, stop=True)

            if kdebug == "convs":
                dbg = zpool.tile([128, NL], F32, tag="zT")
                nc.vector.tensor_copy(dbg[:], fc_ps[:])
                nc.sync.dma_start(out_d[0:128, 0:NL], dbg[:])
            else:
                _emit_tail(nc, tc, zpool, fc_ps, fcb, bna, bnb, abl, abf,
                           onc_d, onr_d, out_d, gin_d, gout_d, kdebug)

    nc.compile()
    return nc


def _emit_tail(nc, tc, zpool, fc_ps, fcb, bna, bnb, abl, abf,
               onc_d, onr_d, out_d, gin_d, gout_d, kdebug):
    I32 = mybir.dt.int32
    with tc.tile_pool(name="sqp", bufs=1, space="PSUM") as sqpool:
        zT = zpool.tile([128, NL], F32, tag="zT")
        nc.scalar.activation(zT[:], fc_ps[:], ACT.Relu,
                             bias=fcb[:], scale=1.0)
        nc.vector.tensor_scalar(zT[:], zT[:], bna[:], bnb[:],
                                op0=AL.mult, op1=AL.add)
        # truncate z to fp22 (e8m13) so the fp32r z.z matmul is consistent
        # with sq computed from the same values
        nc.vector.tensor_scalar(zT[:].bitcast(I32), zT[:].bitcast(I32),
                                -1024, None, op0=AL.bitwise_and,
                                op1=AL.bypass)
        zsq = zpool.tile([128, NL], F32, tag="zsq")
        nc.scalar.activation(zsq[:], zT[:], ACT.Square)
        ones_col = zpool.tile([128, 1], F32, tag="ones_col")
        nc.sync.dma_start(ones_col[:], onc_d[:])
        sq_ps = sqpool.tile([1, NL], F32, tag="sq")
        nc.tensor.matmul(sq_ps[:], ones_col[:], zsq[:],
                         start=True, stop=True)
        sqones = zpool.tile([2, NL], F32, tag="sqones")
        nc.sync.dma_start(sqones[1:2, :], onr_d[0:1, 0:NL])
        nc.vector.tensor_copy(sqones[0:1, :], sq_ps[:])
        zm2 = zpool.tile([128, NL], F32R, tag="zm2")
        nc.vector.tensor_scalar_mul(zm2[:], zT[:], -2.0)

        if kdebug == "z":
            nc.sync.dma_start(out_d[0:128, 0:NL], zT[:])
            return

        nc.sync.dma_start(gin_d[0:128, :], zT[:])
        nc.sync.dma_start(gin_d[128:129, :], sqones[0:1, :])
        nc.gpsimd.collective_compute(
            "AllGather", AL.bypass,
            replica_groups=[list(range(N_CORES))],
            ins=[gin_d[:]], outs=[gout_d[:]],
        )

        zfT = zpool.tile([128, N_CORES, NL], F32R, tag="zfT")
        nc.sync.dma_start(
            zfT[:],
            gout_d[:, 0:128, :].rearrange("r p n -> p r n").bitcast(F32R))
        onesqf = zpool.tile([2, N], F32, tag="onesqf")
        nc.sync.dma_start(onesqf[0:1, :], onr_d[:])
        nc.sync.dma_start(
            onesqf[1:2, :].rearrange("p (r n) -> p r n", r=N_CORES),
            gout_d[:, 128:129, :].rearrange("r p n -> p r n"))

        if kdebug == "gather":
            zfc = zpool.tile([128, NL], F32, tag="zfc")
            nc.vector.tensor_copy(zfc[:], zfT[:, 0, :])
            nc.sync.dma_start(out_d[0:128, 0:NL], zfc[:])
            return

        # ---- loss row block ----
        with (
            tc.tile_pool(name="pd", bufs=2, space="PSUM") as pdpool,
            tc.tile_pool(name="py", bufs=2, space="PSUM") as pypool,
            tc.tile_pool(name="lw", bufs=4) as lwpool,
        ):
            for rb in range(4):
                rs = slice(128 * rb, 128 * rb + 128)
                for jc in range(N_CORES):
                    js = slice(NL * jc, NL * jc + NL)
                    pd = pdpool.tile([128, NL], F32, tag="pd")
                    py = pypool.tile([128, NL], F32, tag="py")
                    nc.tensor.matmul(pd[:], zm2[:, rs], zfT[:, jc, :],
                                     start=True, stop=False)
                    nc.tensor.matmul(pd[:], sqones[:, rs], onesqf[:, js],
                                     start=False, stop=True)
                    nc.tensor.matmul(py[:], abl[:, rs], abf[:, js],
                                     start=True, stop=True)
                    dd = lwpool.tile([128, NL], F32, tag="dd")
                    nc.scalar.activation(dd[:], pd[:], ACT.Sqrt)
                    tt = lwpool.tile([128, NL], F32, tag="tt")
                    nc.scalar.activation(tt[:], dd[:], ACT.Relu,
                                         bias=1.0, scale=-1.0)
                    cl = lwpool.tile([128, NL], F32, tag="cl")
                    nc.vector.select(
                        cl[:], py[:].bitcast(I32), dd[:], tt[:])
                    nc.sync.dma_start(out_d[rs, js], cl[:])


def _prep_inputs(samples, samples_info, conv1_w, conv1_b, conv2_w, conv2_b,
                 conv3_w, conv3_b, fc_w, fc_b, bn_gamma, bn_beta, bn_mean,
                 bn_var):
    f = np.float32
    samples = np.asarray(samples, f)
    info = np.asarray(samples_info, f)
    w1 = np.asarray(conv1_w, f)
    w2 = np.asarray(conv2_w, f)
    w3 = np.asarray(conv3_w, f)
    fcwf = np.asarray(fc_w, f)

    assert np.all(np.asarray(conv1_b) == 0), "conv1_b != 0 unsupported"
    assert np.all(np.asarray(conv2_b) == 0), "conv2_b != 0 unsupported"
    assert np.all(np.asarray(conv3_b) == 0), "conv3_b != 0 unsupported"

    # ---- reference means r_l from a sample subset (fp32 chain) ----
    nsub = 256
    xs_sub = samples[:nsub, 0, :]                       # [S, L]

    def convl(h, w, pad):
        # h [S, C, Lc], w [O, C, K] -> [S, O, Lc]
        S, C, Lc = h.shape
        O, _, K = w.shape
        hp = np.pad(h, ((0, 0), (0, 0), pad))
        cols = np.stack([hp[:, :, k:k + Lc] for k in range(K)], axis=1)
        cols = cols.reshape(S, K * C, Lc)
        wm = w.transpose(0, 2, 1).reshape(O, K * C)
        return np.einsum('ok,nkl->nol', wm, cols, optimize=True)

    def pool2(x):
        n, c, l = x.shape
        return x.reshape(n, c, l // 2, 2).max(-1)

    h1s = np.maximum(pool2(convl(xs_sub[:, None, :], w1, (49, 50))), 0)
    r1 = h1s.mean(axis=(0, 2)).astype(f)                # [64]
    h2s = np.maximum(pool2(convl(h1s, w2, (2, 2))), 0)
    r2 = h2s.mean(axis=(0, 2)).astype(f)                # [128]
    h3s = np.maximum(pool2(convl(h2s, w3, (1, 1))), 0)
    r3 = h3s.mean(axis=(0, 2)).astype(f)                # [256]

    # exact per-position biases conv(r) at the pre-pool level
    # conv2 input length = T1 (pooled conv1), conv3 input length = T2
    b2 = convl(np.broadcast_to(r1[None, :, None], (1, C1, T1)).astype(f),
               w2, (2, 2))[0]                           # [128, T1]
    b3 = convl(np.broadcast_to(r2[None, :, None], (1, C2, T2)).astype(f),
               w3, (1, 1))[0]                           # [256, T2]
    b2i = b2[:, 100]                                    # interior value
    b3i = b3[:, 100]

    # ---- conv1 shifted weights: 16 variants, window <= 118 rows ----
    w1s = np.zeros((16, 128, 128), f)
    for s in range(16):
        w1s[s, s:s + K1, 0:C1] = w1[:, 0, :].T
        w1s[s, s + 2:s + 2 + K1, C1:128] = w1[:, 0, :].T

    # ---- conv2 DR weight variants: v = role*4 + (l2 % 4) ----
    # contraction row (p, t): pooled pi = 4u + 2t + p//64, ic = p % 64
    # tap tau = pi - (l2 - 2); role A: l2-4u = m4+4, role B: l2-4u = m4
    w2v = np.zeros((8, 128, 2, C2), f)
    for role in range(2):
        for m4 in range(4):
            # uA = (l2-2)//4 -> deltaA = l2-4uA = [4,5,2,3][m4]; deltaB-4
            delta = (m4 + 4 if m4 < 2 else m4) - (0 if role == 0 else 4)
            for t in range(2):
                for half in range(2):
                    tau = 2 * t + half - delta + 2
                    if 0 <= tau < K2:
                        w2v[role * 4 + m4, 64 * half:64 * half + 64, t, :] = \
                            w2[:, :, tau].T
    # ---- conv3 DR variants: v = 4*parity + 2*ch + role ----
    # rows = 128 ic; ktile t covers pooled2 pi = 2u + t; tau = pi - (l3-1)
    # even l3 (par 0): role A u=m-1 -> tau = t-1 ; role B u=m -> tau = t+1
    # odd  l3 (par 1): role A u=m   -> tau = t   ; role B u=m+1 -> tau = t+2
    w3v = np.zeros((8, 128, 2, 128), f)
    for par in range(2):
        for role in range(2):
            for ch in range(2):
                for t in range(2):
                    tau = t + [[-1, 1], [0, 2]][par][role]
                    if 0 <= tau < K3:
                        w3v[4 * par + 2 * ch + role, :, t, :] = \
                            w3[128 * ch:128 * ch + 128, :, tau].T

    fcw = np.zeros((2, 128, 128), f)
    fcwT = fcwf.T / f(T3)
    fcw[0] = fcwT[0:128, :]
    fcw[1] = fcwT[128:256, :]
    fcb = np.asarray(fc_b, f).reshape(128, 1)
    bna = (np.asarray(bn_gamma, f) /
           np.sqrt(np.asarray(bn_var, f) + f(1e-5))).reshape(128, 1)
    bnb = (np.asarray(bn_beta, f) -
           np.asarray(bn_mean, f).reshape(128) * bna[:, 0]).reshape(128, 1)

    # ---- cvec ----
    cvec = np.zeros((128, CV_K), f)
    cvec[:, CV_NR1] = -np.tile(r1, 2)
    cvec[:, CV_B2I] = b2i - r2
    cvec[:, CV_NR2] = -r2
    for k, l2 in enumerate((0, 1, T1 - 2, T1 - 1)):
        cvec[:, CV_B2E + k] = b2[:, l2] - r2
    for ch in range(2):
        cs = slice(128 * ch, 128 * ch + 128)
        cvec[:, CV_B3I + ch] = b3i[cs]
        for k, l3 in enumerate((0, 1, T2 - 2, T2 - 1)):
            cvec[:, CV_B3E + 4 * ch + k] = b3[cs, l3]

    writer, gen = info[:, 0], info[:, 1]
    assert np.all((writer == 0) | (writer == 1)), "non-binary writer id"
    a_full = (gen * (1.0 - writer)).astype(f)
    b_full = (gen * writer).astype(f)
    abf = np.stack([a_full, b_full])

    w1s_b = w1s.astype(BFNP)
    w2v_8 = w2v.astype(E4NP)
    w3v_8 = w3v.astype(E4NP)
    fcw_b = fcw.astype(BFNP)

    ones_col_np = np.ones((128, 1), f)
    ones_row_np = np.ones((1, N), f)

    in_maps = []
    for core in range(N_CORES):
        n0 = core * NL
        xpad = np.zeros((624, NL), f)
        xpad[49:49 + L, :] = samples[n0:n0 + NL, 0, :].T
        xsc = np.zeros((XCH, 128, NL), f)
        for c in range(XCH):
            xsc[c] = xpad[16 * c:16 * c + 128, :]
        in_maps.append({
            "xs": xsc.astype(BFNP), "onc": ones_col_np, "onr": ones_row_np,
            "w1s": w1s_b, "w2v": w2v_8, "w3v": w3v_8, "fcw": fcw_b,
            "fcb": fcb, "bna": bna, "bnb": bnb, "cvec": cvec,
            "abl": np.ascontiguousarray(abf[:, n0:n0 + NL]).astype(BFNP),
            "abf": abf.astype(BFNP),
        })
    return in_maps


def kernel(**inputs):
    global LAST_RESULT
    in_maps = _prep_inputs(**inputs)
    nc = build_nc()
    res = run_bass_kernel_spmd(nc, in_maps, core_ids=list(range(N_CORES)))
    LAST_RESULT = res
    out = np.concatenate([r["out"] for r in res.results], axis=0)
    np.fill_diagonal(out, 0.0)
    return out.astype(np.float32)
